# revision 1
# baseline (speedup 1.0000x reference)
"""Self-contained Trainium2 Bass kernel for nn_ConfigurableGAT
(3-layer GAT, N=50000, E=800000, 8 NeuronCores). Optimized v2.

vs v1: batched dma_gathers (9 chunks per gather), bf16 dense path with
DMA-transpose activations, per-block-fused DVE ops (batched S build,
logits, exp, rhs), transpose-trick Sd matrices, AllGather interleaved
with the dense phase.
"""
import numpy as np
from contextlib import ExitStack

import jax
import concourse.bass as bass
import concourse.bacc as bacc
import concourse.mybir as mybir
import concourse.tile as tile
from concourse.library_config import mlp
from concourse import bass2jax
from concourse.bass2jax import _bass_exec_p, install_neuronx_cc_hook
from jax.sharding import Mesh, PartitionSpec
try:
    from jax.experimental.shard_map import shard_map
except ImportError:
    from jax.sharding import shard_map


N_CORES = 8
P = 128


def plan_graph(edge_index, n_nodes=50000, n_cores=N_CORES, k_lo=9, k_hi=9):
    src = np.asarray(edge_index[0], dtype=np.int64)
    dst = np.asarray(edge_index[1], dtype=np.int64)
    E = src.shape[0]

    deg = np.bincount(dst, minlength=n_nodes)   # random edges only
    n_blocks_total = -(-n_nodes // P)
    n_blocks_total = -(-n_blocks_total // n_cores) * n_cores
    n_blocks = n_blocks_total // n_cores
    NP = n_blocks_total * P
    half = NP // 2
    K = k_lo + k_hi

    # ---- stage 1: assign nodes to cores, balancing total in-degree.
    order = np.argsort(-deg, kind="stable")
    core_fill = np.zeros(n_cores, dtype=np.int64)
    core_cnt = np.zeros(n_cores, dtype=np.int64)
    node_core = np.full(n_nodes, -1, dtype=np.int64)
    npc = n_blocks * P
    for nid in order:
        c = np.argmin(np.where(core_cnt < npc, core_fill, np.iinfo(np.int64).max))
        node_core[nid] = c
        core_fill[c] += deg[nid]
        core_cnt[c] += 1

    src_is_lo = node_core[src] < n_cores // 2
    deg_lo = np.bincount(dst[src_is_lo], minlength=n_nodes)
    deg_hi = deg - deg_lo

    # ---- stage 2: per core, pack nodes into blocks with dual caps.
    cap_lo, cap_hi = k_lo * P, k_hi * P
    node_block = np.full(n_nodes, -1, dtype=np.int64)
    for c in range(n_cores):
        nodes = np.where(node_core == c)[0]
        nodes = nodes[np.argsort(-(deg_lo[nodes] + deg_hi[nodes]), kind="stable")]
        bl = np.zeros(n_blocks, dtype=np.int64)
        bh = np.zeros(n_blocks, dtype=np.int64)
        bc = np.zeros(n_blocks, dtype=np.int64)
        for nid in nodes:
            dl, dh = deg_lo[nid], deg_hi[nid]
            ok = (bl + dl <= cap_lo) & (bh + dh <= cap_hi) & (bc < P)
            if not ok.any():
                raise RuntimeError(
                    f"packing failed core {c}: need k_lo/k_hi larger "
                    f"(deg {dl}/{dh}, fills {bl.max()}/{bh.max()})")
            cand = np.where(ok)[0]
            util = np.maximum((bl[cand] + dl) / cap_lo, (bh[cand] + dh) / cap_hi)
            b = cand[np.argmin(util)]
            node_block[nid] = c * n_blocks + b
            bl[b] += dl
            bh[b] += dh
            bc[b] += 1

    # ---- permutation
    perm = np.full(NP, -1, dtype=np.int64)
    inv = np.full(n_nodes, -1, dtype=np.int64)
    fill_cnt = np.zeros(n_blocks_total, dtype=np.int64)
    for nid in range(n_nodes):
        b = node_block[nid]
        slot = b * P + fill_cnt[b]
        fill_cnt[b] += 1
        perm[slot] = nid
        inv[nid] = slot

    # ---- edge layout
    psrc = inv[src]
    pdst = inv[dst]
    pblock = pdst // P
    is_lo = psrc < half
    order_e = np.lexsort((psrc, pdst, ~is_lo, pblock))
    psrc_s = psrc[order_e]
    pdst_s = pdst[order_e]
    pblock_s = pblock[order_e]
    islo_s = is_lo[order_e]

    idx16 = np.zeros((n_blocks_total, K * P), np.int16)
    seg = np.full((n_blocks_total, K * P), -1.0, np.float32)
    starts = np.searchsorted(pblock_s, np.arange(n_blocks_total))
    ends = np.searchsorted(pblock_s, np.arange(n_blocks_total), side="right")
    for b in range(n_blocks_total):
        s, e = starts[b], ends[b]
        lo_mask = islo_s[s:e]
        nlo = int(lo_mask.sum())
        nhi = (e - s) - nlo
        assert nlo <= cap_lo and nhi <= cap_hi, (b, nlo, nhi)
        idx16[b, :nlo] = psrc_s[s:s + nlo]
        seg[b, :nlo] = (pdst_s[s:s + nlo] - b * P)
        idx16[b, cap_lo:cap_lo + nhi] = psrc_s[s + nlo:e] - half
        seg[b, cap_lo:cap_lo + nhi] = (pdst_s[s + nlo:e] - b * P)

    return dict(perm=perm, inv=inv, idx16=idx16, seg=seg, K=K, k_lo=k_lo,
                k_hi=k_hi, n_blocks=n_blocks, NP=NP, half=half,
                n_chunks=n_blocks * K)


def pack_inputs(plan, x, weights, n_cores=N_CORES):
    """Build the per-core input dicts for the bass kernel."""
    import ml_dtypes
    BF = ml_dtypes.bfloat16
    n_blocks, K = plan["n_blocks"], plan["K"]
    NP, perm = plan["NP"], plan["perm"]
    NPC = n_blocks * P
    n_chunks = n_blocks * K
    IN_C = np.asarray(x).shape[1]

    xp = np.zeros((NP, IN_C), np.float32)
    valid = perm >= 0
    xp[valid] = np.asarray(x, np.float32)[perm[valid]]
    xp = xp.astype(BF)

    def wcat(W, a_s, a_d):
        W = np.asarray(W, np.float32)
        IF, F = W.shape
        a_s = np.asarray(a_s, np.float32)
        a_d = np.asarray(a_d, np.float32)
        H, C = a_s.shape
        As = np.zeros((F, H), np.float32)
        Ad = np.zeros((F, H), np.float32)
        for h in range(H):
            As[h * C:(h + 1) * C, h] = a_s[h]
            Ad[h * C:(h + 1) * C, h] = a_d[h]
        Wc = np.concatenate([W, W @ As, W @ Ad], axis=1)
        KT = IF // 128
        return np.ascontiguousarray(
            Wc.reshape(KT, 128, F + 2 * H).transpose(1, 0, 2)).astype(BF)

    w1 = wcat(weights["W1"], weights["a_src1"], weights["a_dst1"])
    w2 = wcat(weights["W2"], weights["a_src2"], weights["a_dst2"])
    w3 = wcat(weights["W3"], weights["a_src3"], weights["a_dst3"])
    b1 = np.broadcast_to(np.asarray(weights["b1"], np.float32), (128, 256)).copy()
    b2 = np.broadcast_to(np.asarray(weights["b2"], np.float32), (128, 256)).copy()
    b3 = np.broadcast_to(np.asarray(weights["b3"], np.float32), (128, 64)).copy()

    # idx wrapped for dma_gather, one wrap per GATHER GROUP (must mirror
    # gat_bass.gather_groups): elem i of group -> [16k + i%16, g0*8 + i//16]
    k_lo, k_hi = plan["k_lo"], plan["k_hi"]
    gb_sz = plan.get("gather_batch", 1)
    groups = []
    for base, cnt in ((0, k_lo), (k_lo, k_hi)):
        j = 0
        while j < cnt:
            n = min(gb_sz, cnt - j)
            groups.append((base + j, n))
            j += n
    idx_in = np.zeros((n_cores, 128, n_chunks * 8), np.int16)
    seg_in = np.zeros((n_cores, 128, n_chunks), np.float32)
    for c in range(n_cores):
        for blk in range(n_blocks):
            gb = c * n_blocks + blk
            for (j0, ng) in groups:
                flat = plan["idx16"][gb, j0 * P:(j0 + ng) * P]
                wrap = flat.reshape(ng * 8, 16).T     # [16, ng*8]
                g0 = blk * K + j0
                idx_in[c, :, g0 * 8:g0 * 8 + ng * 8] = np.tile(wrap, (8, 1))
            for j in range(K):
                g = blk * K + j
                seg_in[c, :, g] = plan["seg"][gb, j * P:(j + 1) * P]
    iotap = np.arange(128, dtype=np.float32)[:, None].copy()
    iotakf = np.broadcast_to(
        np.tile(np.arange(128, dtype=np.float32), K), (128, K * 128)).copy()
    ins = []
    for c in range(n_cores):
        ins.append({
            "x": xp[c * NPC:(c + 1) * NPC].copy(),
            "idx": idx_in[c],
            "seg": seg_in[c],
            "wcat1": w1, "wcat2": w2, "wcat3": w3,
            "bias1": b1, "bias2": b2, "bias3": b3,
            "iden": np.eye(128, dtype=np.float32),
            "iotap": iotap,
            "iotakf": iotakf,
            "asrc1": _asrc_flat(weights["a_src1"], BF),
            "asrc2": _asrc_flat(weights["a_src2"], BF),
            "asrc3": _asrc_flat(weights["a_src3"], np.float32),
        })
    return ins


def _asrc_flat(a, dt):
    flat = np.asarray(a, np.float32).reshape(-1)
    return np.broadcast_to(flat, (128, flat.shape[0])).astype(dt).copy()


def unpack_output(plan, outs, n_nodes=50000):
    perm, NP = plan["perm"], plan["NP"]
    full = np.concatenate([o["out"] for o in outs], axis=0)
    assert full.shape[0] == NP
    valid = perm >= 0
    res = np.zeros((n_nodes, full.shape[1]), np.float32)
    res[perm[valid]] = full[valid]
    return res


F32 = mybir.dt.float32
BF16 = mybir.dt.bfloat16
AF = mybir.ActivationFunctionType
OP = mybir.AluOpType


def gather_groups(k_lo, k_hi, gb):
    groups = []
    for base, cnt in ((0, k_lo), (k_lo, k_hi)):
        j = 0
        while j < cnt:
            n = min(gb, cnt - j)
            groups.append((base + j, n, base == 0))
            j += n
    return groups


USE_LRELU = False
SKIP_AG = False      # timing-probe only: drop the AllGathers
SKIP_GATHER = False  # timing-probe only: drop the dma_gathers


def build_gat_v2(n_blocks: int, k_lo: int, k_hi: int, n_cores: int = 8,
                 in_feat: int = 256, gather_batch: int = 9,
                 ag_groups: int = 7):
    """Input tensor names (per core):
      x      [NPC, in_feat] bf16
      idx    [128, n_chunks*8] int16  (wrapped+replicated dma_gather indices)
      seg    [128, n_chunks] f32      (local dst 0..127, -1 pad)
      wcat1  [128, in_feat//128, 272] bf16; wcat2 [128,2,272]; wcat3 [128,2,66]
      asrc1, asrc2 [128, 256] bf16 replicated a_src flat; asrc3 [128, 64] f32
      bias1, bias2 [128, 256] f32; bias3 [128, 64] f32
      iden [128,128] f32; iotap [128,1] f32; iotakf [128, K*128] f32
    Output: out [NPC, 64] f32
    """
    P = 128
    K = k_lo + k_hi
    NPC = n_blocks * P
    NP = NPC * n_cores
    half = NP // 2
    n_chunks = n_blocks * K
    assert n_blocks % ag_groups == 0
    grp_blocks = n_blocks // ag_groups

    nc = bacc.Bacc("TRN2", target_bir_lowering=False, debug=False,
                   num_devices=n_cores)

    x_in = nc.dram_tensor("x", [NPC, in_feat], BF16, kind="ExternalInput")
    idx_in = nc.dram_tensor("idx", [128, n_chunks * 8], mybir.dt.int16,
                            kind="ExternalInput")
    seg_in = nc.dram_tensor("seg", [128, n_chunks], F32, kind="ExternalInput")
    kt1 = in_feat // 128
    wcat_in = [
        nc.dram_tensor("wcat1", [128, kt1, 272], BF16, kind="ExternalInput"),
        nc.dram_tensor("wcat2", [128, 2, 272], BF16, kind="ExternalInput"),
        nc.dram_tensor("wcat3", [128, 2, 66], BF16, kind="ExternalInput"),
    ]
    asrc_in = [
        nc.dram_tensor("asrc1", [128, 256], BF16, kind="ExternalInput"),
        nc.dram_tensor("asrc2", [128, 256], BF16, kind="ExternalInput"),
        nc.dram_tensor("asrc3", [128, 64], F32, kind="ExternalInput"),
    ]
    bias_in = [
        nc.dram_tensor("bias1", [128, 256], F32, kind="ExternalInput"),
        nc.dram_tensor("bias2", [128, 256], F32, kind="ExternalInput"),
        nc.dram_tensor("bias3", [128, 64], F32, kind="ExternalInput"),
    ]
    iden_in = nc.dram_tensor("iden", [128, 128], F32, kind="ExternalInput")
    iotap_in = nc.dram_tensor("iotap", [128, 1], F32, kind="ExternalInput")
    iotakf_in = nc.dram_tensor("iotakf", [128, K * 128], F32,
                               kind="ExternalInput")
    out_ext = nc.dram_tensor("out", [NPC, 64], F32, kind="ExternalOutput")

    # (F, H, C, in_feat, table dtype)
    LAY = [(256, 8, 32, in_feat, BF16), (256, 8, 32, 256, BF16),
           (64, 1, 64, 256, F32)]

    T_loc = [nc.dram_tensor(f"Tloc{i}", [NPC, LAY[i][0]], LAY[i][4])
             for i in range(3)]
    T_full = [nc.dram_tensor(f"Tfull{i}", [NP, LAY[i][0]], LAY[i][4],
                             addr_space="Shared")
              for i in range(3)]
    T_comp = [nc.dram_tensor(f"Tcomp{i}", [NP, LAY[i][0]], LAY[i][4],
                             addr_space="Shared")
              for i in range(3)]
    act_d = [None,
             nc.dram_tensor("act2", [NPC, 256], BF16),
             nc.dram_tensor("act3", [NPC, 256], BF16)]

    replica_groups = [list(range(n_cores))]

    with ExitStack() as ctx:
        tc = ctx.enter_context(tile.TileContext(nc))
        const = ctx.enter_context(tc.tile_pool(name="const", bufs=1))
        sb = ctx.enter_context(tc.tile_pool(name="sb", bufs=2))
        sb3 = ctx.enter_context(tc.tile_pool(name="sb3", bufs=3))
        ps = ctx.enter_context(tc.tile_pool(name="ps", bufs=2, space="PSUM"))

        nc.gpsimd.load_library(mlp)

        ident = const.tile([128, 128], F32)
        nc.sync.dma_start(out=ident[:], in_=iden_in[:])
        ident_b = const.tile([128, 128], BF16)
        nc.vector.tensor_copy(out=ident_b[:], in_=ident[:])
        iotap = const.tile([128, 1], F32)
        nc.sync.dma_start(out=iotap[:], in_=iotap_in[:])
        iotakf = const.tile([128, K * 128], F32)
        nc.sync.dma_start(out=iotakf[:], in_=iotakf_in[:])
        idx_t = const.tile([128, n_chunks * 8], mybir.dt.int16)
        nc.sync.dma_start(out=idx_t[:], in_=idx_in[:])
        seg_t = const.tile([128, n_chunks], F32)
        nc.sync.dma_start(out=seg_t[:], in_=seg_in[:])

        for li in range(3):
            F, H, C, IF, TDl = LAY[li]
            KT = IF // 128
            C2 = F + 2 * H
            KH = (K + 1) * H
            act_ap = x_in if li == 0 else act_d[li]
            Tf = T_full[li]
            Tf3 = Tf[:].rearrange("(r n) c -> r n c", r=n_cores)

            # ---------------- per-layer constants
            wc = sb.tile([128, KT, C2], BF16, tag="wc")
            nc.sync.dma_start(out=wc[:], in_=wcat_in[li][:])
            bia = sb.tile([128, F], F32, tag="bias")
            nc.sync.dma_start(out=bia[:], in_=bias_in[li][:])
            asr = sb.tile([128, F], TDl, tag="asrc")
            nc.sync.dma_start(out=asr[:], in_=asrc_in[li][:])
            asr_rep = sb.tile([128, K + 1, F], TDl, tag="asrep")
            nc.vector.tensor_copy(
                out=asr_rep[:],
                in_=asr[:, None, :].to_broadcast([128, K + 1, F]))
            al_d_all = sb.tile([128, n_blocks, H], TDl, tag=f"ald{li}")

            # ---------------- dense phase + interleaved AllGather
            for rb in range(n_blocks):
                att = sb3.tile([128, KT, 128], BF16, tag="att")
                for kt in range(KT):
                    nc.scalar.dma_start_transpose(
                        out=att[:, kt, :],
                        in_=act_ap[rb * P:(rb + 1) * P, kt * 128:(kt + 1) * 128])
                pd = ps.tile([128, C2], F32, tag="pdense")
                for kt in range(KT):
                    nc.tensor.matmul(pd[:], lhsT=att[:, kt, :], rhs=wc[:, kt, :],
                                     start=(kt == 0), stop=(kt == KT - 1))
                trow = sb3.tile([128, F], TDl, tag="trow")
                nc.vector.tensor_copy(out=trow[:], in_=pd[:, :F])
                nc.sync.dma_start(out=T_loc[li][rb * P:(rb + 1) * P, :],
                                  in_=trow[:])
                nc.vector.tensor_copy(out=al_d_all[:, rb, :], in_=pd[:, F + H:])
                if (rb + 1) % grp_blocks == 0 and not SKIP_AG:
                    r0 = (rb + 1 - grp_blocks) * P
                    r1 = (rb + 1) * P
                    # AllGather into a contiguous group-major slab, then two
                    # DMAs (sync + scalar queues) un-interleave the slab into
                    # the standard-order gather table; overlaps with the rest
                    # of the dense phase
                    gr = r1 - r0
                    slab = T_comp[li][n_cores * r0:n_cores * r1, :]
                    slab3 = slab.rearrange("(r n) c -> r n c", r=n_cores)
                    nc.gpsimd.collective_compute(
                        "AllGather", OP.bypass,
                        replica_groups=replica_groups,
                        ins=[T_loc[li][r0:r1, :]],
                        outs=[slab3],
                    )
                    rm = r0 + gr // 2
                    nc.sync.dma_start(out=Tf3[:, r0:rm, :],
                                      in_=slab3[:, :gr // 2, :])
                    nc.scalar.dma_start(out=Tf3[:, rm:r1, :],
                                        in_=slab3[:, gr // 2:, :])

            # ---------------- edge phase
            for b in range(n_blocks):
                G_all = sb.tile([128, K + 1, F], TDl, tag="G")
                for gi, (j0, ng, is_lo) in enumerate(
                        [] if SKIP_GATHER else
                        gather_groups(k_lo, k_hi, gather_batch)):
                    g0 = b * K + j0
                    src_view = Tf[:half, :] if is_lo else Tf[half:, :]
                    nc.gpsimd.dma_gather(
                        out_ap=G_all[:, j0:j0 + ng, :],
                        in_ap=src_view,
                        idxs_ap=idx_t[:, g0 * 8:g0 * 8 + ng * 8],
                        num_idxs=ng * 128,
                        num_idxs_reg=ng * 128,
                        elem_size=F,
                        queue_num=0,
                    )
                # self rows as chunk K
                nc.sync.dma_start(out=G_all[:, K, :],
                                  in_=T_loc[li][b * P:(b + 1) * P, :])

                # S (edge-major one-hot) for all K chunks in one op
                S_all = sb.tile([128, K, 128], TDl, tag="S")
                nc.vector.tensor_tensor(
                    out=S_all[:],
                    in0=iotakf[:].rearrange("p (k d) -> p k d", k=K),
                    in1=seg_t[:, b * K:(b + 1) * K, None].to_broadcast(
                        [128, K, 128]),
                    op=OP.is_equal)

                # Sd (dst-major one-hot) via transpose trick, 4 chunks/bank;
                # ald matmuls accumulate into one psum tile
                ald_ps = ps.tile([128, K * H], F32, tag="aldps")
                for g4 in range(0, K, 4):
                    n4 = min(4, K - g4)
                    pt = ps.tile([128, 4, 128], F32, tag="ptr")
                    for i in range(n4):
                        g = b * K + g4 + i
                        nc.tensor.transpose(
                            pt[:, i, :],
                            seg_t[:, g:g + 1].to_broadcast([128, 128]),
                            ident[:])
                    Sd4 = sb3.tile([128, 4, 128], TDl, tag="Sd")
                    nc.vector.tensor_scalar(
                        out=Sd4[:, :n4, :], in0=pt[:, :n4, :],
                        scalar1=iotap[:, :], scalar2=None, op0=OP.is_equal)
                    for i in range(n4):
                        j = g4 + i
                        nc.tensor.matmul(
                            ald_ps[:, j * H:(j + 1) * H],
                            lhsT=Sd4[:, i, :], rhs=al_d_all[:, b, :],
                            start=True, stop=True)

                ald_all = sb3.tile([128, KH], F32, tag="alda")
                nc.vector.tensor_copy(out=ald_all[:, :K * H], in_=ald_ps[:])
                nc.vector.tensor_copy(out=ald_all[:, K * H:],
                                      in_=al_d_all[:, b, :])

                # al_s for all chunks (incl self) in two ops
                tmp = sb3.tile([128, (K + 1) * F], TDl, tag="alstmp")
                nc.vector.tensor_tensor(
                    out=tmp[:], in0=G_all[:].rearrange("p k f -> p (k f)"),
                    in1=asr_rep[:].rearrange("p k f -> p (k f)"), op=OP.mult)
                als_all = sb3.tile([128, KH], F32, tag="alsa")
                nc.vector.tensor_reduce(
                    out=als_all[:],
                    in_=tmp[:].rearrange("p (g c) -> p g c", c=C),
                    axis=mybir.AxisListType.X, op=OP.add)

                # logits -> leaky relu -> exp  (batched)
                lg = sb3.tile([128, KH], F32, tag="lg")
                nc.vector.tensor_tensor(out=lg[:], in0=ald_all[:],
                                        in1=als_all[:], op=OP.add)
                lt = sb3.tile([128, KH], F32, tag="lt")
                if USE_LRELU:
                    nc.scalar.activation(out=lt[:], in_=lg[:], func=AF.Lrelu,
                                         alpha=0.2)
                else:
                    nc.vector.tensor_scalar_mul(lt[:], lg[:], 0.2)
                    nc.vector.tensor_tensor(out=lt[:], in0=lt[:], in1=lg[:],
                                            op=OP.max)
                exb = sb3.tile([128, KH], TDl, tag="exb")
                nc.scalar.activation(out=exb[:], in_=lt[:], func=AF.Exp)

                # rhs = [G*ex | ex]
                rhs = sb.tile([128, K + 1, H, C + 1], TDl, tag="rhs")
                exv = exb[:].rearrange("p (k h) -> p k h", h=H)
                nc.vector.tensor_tensor(
                    out=rhs[:, :, :, :C],
                    in0=G_all[:].rearrange("p k (h c) -> p k h c", h=H),
                    in1=exv[:, :, :, None].to_broadcast([128, K + 1, H, C]),
                    op=OP.mult)
                nc.vector.tensor_copy(out=rhs[:, :, :, C], in_=exv)

                # aggregation
                pa = ps.tile([128, H * (C + 1)], F32, tag="pagg")
                for j in range(K + 1):
                    if j < K:
                        lhs_agg = S_all[:, j, :]
                    else:
                        lhs_agg = ident_b[:] if TDl == BF16 else ident[:]
                    nc.tensor.matmul(
                        pa[:], lhsT=lhs_agg,
                        rhs=rhs[:, j].rearrange("p h c -> p (h c)"),
                        start=(j == 0), stop=(j == K))

                # normalize + bias (+ELU)
                pa3 = pa[:].rearrange("p (h c) -> p h c", h=H)
                dn = sb3.tile([128, H], F32, tag="dn")
                nc.vector.tensor_scalar_add(dn[:], pa3[:, :, C], 1e-30)
                rc = sb3.tile([128, H], F32, tag="rc")
                nc.vector.reciprocal(rc[:], dn[:])
                ob = sb3.tile([128, F], F32, tag="ob")
                ob3 = ob[:].rearrange("p (h c) -> p h c", h=H)
                nc.vector.tensor_tensor(out=ob3, in0=pa3[:, :, :C],
                                        in1=rc[:, :, None].to_broadcast([128, H, C]),
                                        op=OP.mult)
                nc.vector.tensor_tensor(out=ob[:], in0=ob[:], in1=bia[:],
                                        op=OP.add)
                if li < 2:
                    # elu(x) = max(x,0) - 1 + exp(min(x,0))
                    mn = sb3.tile([128, F], F32, tag="mn")
                    nc.vector.tensor_scalar_min(mn[:], ob[:], 0.0)
                    em = sb3.tile([128, F], F32, tag="em")
                    nc.scalar.activation(out=em[:], in_=mn[:], func=AF.Exp)
                    acb = sb3.tile([128, F], F32, tag="acb")
                    nc.vector.tensor_scalar(
                        out=acb[:], in0=ob[:], scalar1=0.0, scalar2=-1.0,
                        op0=OP.max, op1=OP.add)
                    actb = sb3.tile([128, F], BF16, tag="actb")
                    nc.vector.tensor_tensor(out=actb[:], in0=acb[:], in1=em[:],
                                            op=OP.add)
                    nc.sync.dma_start(
                        out=act_d[li + 1][b * P:(b + 1) * P, :], in_=actb[:])
                else:
                    nc.sync.dma_start(out=out_ext[b * P:(b + 1) * P, :],
                                      in_=ob[:])

    nc.compile()
    return nc


class SpmdRunner:
    def __init__(self, nc, n_cores: int):
        install_neuronx_cc_hook()
        self.nc = nc
        self.n_cores = n_cores
        in_names, out_names, out_avals, zero_outs = [], [], [], []
        partition_name = nc.partition_id_tensor.name if nc.partition_id_tensor else None
        for alloc in nc.m.functions[0].allocations:
            if not isinstance(alloc, mybir.MemoryLocationSet):
                continue
            name = alloc.memorylocations[0].name
            if alloc.kind == "ExternalInput":
                if name != partition_name:
                    in_names.append(name)
            elif alloc.kind == "ExternalOutput":
                out_names.append(name)
                shape = tuple(alloc.tensor_shape)
                dtype = mybir.dt.np(alloc.dtype)
                out_avals.append(jax.core.ShapedArray(shape, dtype))
                zero_outs.append(np.zeros(shape, dtype))
        self.in_names, self.out_names = in_names, out_names
        self.out_avals, self.zero_outs = out_avals, zero_outs
        n_params = len(in_names)
        n_outs = len(out_avals)
        all_in_names = list(in_names) + list(out_names)
        if partition_name is not None:
            all_in_names.append(partition_name)

        def _body(*args):
            operands = list(args)
            if partition_name is not None:
                operands.append(bass2jax.partition_id_tensor())
            outs = _bass_exec_p.bind(
                *operands,
                out_avals=tuple(out_avals),
                in_names=tuple(all_in_names),
                out_names=tuple(out_names),
                lowering_input_output_aliases=(),
                sim_require_finite=True,
                sim_require_nnan=True,
                nc=nc,
            )
            return tuple(outs)

        devices = jax.devices()[:n_cores]
        self.mesh = Mesh(np.asarray(devices), ("core",))
        in_specs = (PartitionSpec("core"),) * (n_params + n_outs)
        out_specs = (PartitionSpec("core"),) * n_outs
        self.fn = jax.jit(
            shard_map(_body, mesh=self.mesh, in_specs=in_specs,
                      out_specs=out_specs, check_rep=False),
            keep_unused=True,
        )
        self.dev_in = None

    def set_inputs(self, in_maps):
        concat_in = [
            np.concatenate([np.asarray(in_maps[c][name]) for c in range(self.n_cores)], axis=0)
            for name in self.in_names
        ]
        concat_zeros = [
            np.zeros((self.n_cores * z.shape[0], *z.shape[1:]), z.dtype)
            for z in self.zero_outs
        ]
        sharding = jax.sharding.NamedSharding(self.mesh, PartitionSpec("core"))
        self.dev_in = [jax.device_put(a, sharding) for a in concat_in + concat_zeros]

    def __call__(self):
        outs = self.fn(*self.dev_in)
        jax.block_until_ready(outs)
        return outs

    def results(self, outs):
        per_core = []
        for c in range(self.n_cores):
            d = {}
            for i, name in enumerate(self.out_names):
                full = np.asarray(outs[i])
                sh = self.out_avals[i].shape
                d[name] = full.reshape(self.n_cores, *sh)[c]
            per_core.append(d)
        return per_core


# ======================================================================
# kernel() entry point
# ======================================================================

_CACHE = {}
_RUNNER = {}

N_NODES = 50000
K_TRY = [(9, 9), (10, 10), (12, 12)]
GATHER_BATCH = 3
AG_GROUPS = 7


def _get_compiled(n_blocks, k_lo, k_hi):
    key = (n_blocks, k_lo, k_hi)
    if key not in _CACHE:
        _CACHE[key] = build_gat_v2(n_blocks=n_blocks, k_lo=k_lo, k_hi=k_hi,
                                   gather_batch=min(GATHER_BATCH, k_lo),
                                   ag_groups=AG_GROUPS if n_blocks % AG_GROUPS == 0 else 1)
    return _CACHE[key]


def _plan_any(edge_index, n_nodes):
    last = None
    for k_lo, k_hi in K_TRY:
        try:
            plan = plan_graph(edge_index, n_nodes=n_nodes, k_lo=k_lo, k_hi=k_hi)
            plan["gather_batch"] = min(GATHER_BATCH, k_lo)
            return plan
        except RuntimeError as e:
            last = e
    raise last


def kernel(**inputs):
    x = np.asarray(inputs["x"])
    edge_index = np.asarray(inputs["edge_index"])
    n_nodes = x.shape[0]
    weights = {k: np.asarray(v) for k, v in inputs.items()
               if k not in ("x", "edge_index")}

    plan = _plan_any(edge_index, n_nodes)
    ins = pack_inputs(plan, x, weights)

    key = (plan["n_blocks"], plan["k_lo"], plan["k_hi"])
    nc = _get_compiled(*key)
    if key not in _RUNNER:
        _RUNNER[key] = SpmdRunner(nc, 8)
    r = _RUNNER[key]
    r.set_inputs(ins)
    outs = r()
    res = unpack_output(plan, r.results(outs), n_nodes=n_nodes)
    return res.astype(np.float32)



# revision 5
# speedup vs baseline: 1.1774x; 1.1774x over previous
"""Self-contained Trainium2 Bass kernel for nn_ConfigurableGAT
(3-layer GAT, N=50000, E=800000, 8 NeuronCores). Optimized v2.

vs v1: batched dma_gathers (9 chunks per gather), bf16 dense path with
DMA-transpose activations, per-block-fused DVE ops (batched S build,
logits, exp, rhs), transpose-trick Sd matrices, AllGather interleaved
with the dense phase.
"""
import numpy as np
from contextlib import ExitStack

import jax
import concourse.bass as bass
import concourse.bacc as bacc
import concourse.mybir as mybir
import concourse.tile as tile
from concourse.library_config import mlp
from concourse import bass2jax
from concourse.bass2jax import _bass_exec_p, install_neuronx_cc_hook
from jax.sharding import Mesh, PartitionSpec
try:
    from jax.experimental.shard_map import shard_map
except ImportError:
    from jax.sharding import shard_map


N_CORES = 8
P = 128


def plan_graph(edge_index, n_nodes=50000, n_cores=N_CORES, k_lo=9, k_hi=9):
    src = np.asarray(edge_index[0], dtype=np.int64)
    dst = np.asarray(edge_index[1], dtype=np.int64)
    E = src.shape[0]

    deg = np.bincount(dst, minlength=n_nodes)   # random edges only
    n_blocks_total = -(-n_nodes // P)
    n_blocks_total = -(-n_blocks_total // n_cores) * n_cores
    n_blocks = n_blocks_total // n_cores
    NP = n_blocks_total * P
    half = NP // 2
    K = k_lo + k_hi

    # ---- stage 1: assign nodes to cores, balancing total in-degree.
    order = np.argsort(-deg, kind="stable")
    core_fill = np.zeros(n_cores, dtype=np.int64)
    core_cnt = np.zeros(n_cores, dtype=np.int64)
    node_core = np.full(n_nodes, -1, dtype=np.int64)
    npc = n_blocks * P
    for nid in order:
        c = np.argmin(np.where(core_cnt < npc, core_fill, np.iinfo(np.int64).max))
        node_core[nid] = c
        core_fill[c] += deg[nid]
        core_cnt[c] += 1

    src_is_lo = node_core[src] < n_cores // 2
    deg_lo = np.bincount(dst[src_is_lo], minlength=n_nodes)
    deg_hi = deg - deg_lo

    # ---- stage 2: per core, pack nodes into blocks with dual caps.
    cap_lo, cap_hi = k_lo * P, k_hi * P
    node_block = np.full(n_nodes, -1, dtype=np.int64)
    for c in range(n_cores):
        nodes = np.where(node_core == c)[0]
        nodes = nodes[np.argsort(-(deg_lo[nodes] + deg_hi[nodes]), kind="stable")]
        bl = np.zeros(n_blocks, dtype=np.int64)
        bh = np.zeros(n_blocks, dtype=np.int64)
        bc = np.zeros(n_blocks, dtype=np.int64)
        for nid in nodes:
            dl, dh = deg_lo[nid], deg_hi[nid]
            ok = (bl + dl <= cap_lo) & (bh + dh <= cap_hi) & (bc < P)
            if not ok.any():
                raise RuntimeError(
                    f"packing failed core {c}: need k_lo/k_hi larger "
                    f"(deg {dl}/{dh}, fills {bl.max()}/{bh.max()})")
            cand = np.where(ok)[0]
            util = np.maximum((bl[cand] + dl) / cap_lo, (bh[cand] + dh) / cap_hi)
            b = cand[np.argmin(util)]
            node_block[nid] = c * n_blocks + b
            bl[b] += dl
            bh[b] += dh
            bc[b] += 1

    # ---- permutation
    perm = np.full(NP, -1, dtype=np.int64)
    inv = np.full(n_nodes, -1, dtype=np.int64)
    fill_cnt = np.zeros(n_blocks_total, dtype=np.int64)
    for nid in range(n_nodes):
        b = node_block[nid]
        slot = b * P + fill_cnt[b]
        fill_cnt[b] += 1
        perm[slot] = nid
        inv[nid] = slot

    # ---- edge layout
    psrc = inv[src]
    pdst = inv[dst]
    pblock = pdst // P
    is_lo = psrc < half
    order_e = np.lexsort((psrc, pdst, ~is_lo, pblock))
    psrc_s = psrc[order_e]
    pdst_s = pdst[order_e]
    pblock_s = pblock[order_e]
    islo_s = is_lo[order_e]

    idx16 = np.zeros((n_blocks_total, K * P), np.int16)
    seg = np.full((n_blocks_total, K * P), -1.0, np.float32)
    starts = np.searchsorted(pblock_s, np.arange(n_blocks_total))
    ends = np.searchsorted(pblock_s, np.arange(n_blocks_total), side="right")
    for b in range(n_blocks_total):
        s, e = starts[b], ends[b]
        lo_mask = islo_s[s:e]
        nlo = int(lo_mask.sum())
        nhi = (e - s) - nlo
        assert nlo <= cap_lo and nhi <= cap_hi, (b, nlo, nhi)
        idx16[b, :nlo] = psrc_s[s:s + nlo]
        seg[b, :nlo] = (pdst_s[s:s + nlo] - b * P)
        idx16[b, cap_lo:cap_lo + nhi] = psrc_s[s + nlo:e] - half
        seg[b, cap_lo:cap_lo + nhi] = (pdst_s[s + nlo:e] - b * P)

    return dict(perm=perm, inv=inv, idx16=idx16, seg=seg, K=K, k_lo=k_lo,
                k_hi=k_hi, n_blocks=n_blocks, NP=NP, half=half,
                n_chunks=n_blocks * K)


def pack_inputs(plan, x, weights, n_cores=N_CORES):
    """Build the per-core input dicts for the bass kernel."""
    import ml_dtypes
    BF = ml_dtypes.bfloat16
    n_blocks, K = plan["n_blocks"], plan["K"]
    NP, perm = plan["NP"], plan["perm"]
    NPC = n_blocks * P
    n_chunks = n_blocks * K
    IN_C = np.asarray(x).shape[1]

    xp = np.zeros((NP, IN_C), np.float32)
    valid = perm >= 0
    xp[valid] = np.asarray(x, np.float32)[perm[valid]]
    xp = xp.astype(BF)

    def wcat(W, a_s, a_d):
        W = np.asarray(W, np.float32)
        IF, F = W.shape
        a_s = np.asarray(a_s, np.float32)
        a_d = np.asarray(a_d, np.float32)
        H, C = a_s.shape
        As = np.zeros((F, H), np.float32)
        Ad = np.zeros((F, H), np.float32)
        for h in range(H):
            As[h * C:(h + 1) * C, h] = a_s[h]
            Ad[h * C:(h + 1) * C, h] = a_d[h]
        Wc = np.concatenate([W, W @ As, W @ Ad], axis=1)
        KT = IF // 128
        return np.ascontiguousarray(
            Wc.reshape(KT, 128, F + 2 * H).transpose(1, 0, 2)).astype(BF)

    w1 = wcat(weights["W1"], weights["a_src1"], weights["a_dst1"])
    w2 = wcat(weights["W2"], weights["a_src2"], weights["a_dst2"])
    w3 = wcat(weights["W3"], weights["a_src3"], weights["a_dst3"])
    b1 = np.broadcast_to(np.asarray(weights["b1"], np.float32), (128, 256)).copy()
    b2 = np.broadcast_to(np.asarray(weights["b2"], np.float32), (128, 256)).copy()
    b3 = np.broadcast_to(np.asarray(weights["b3"], np.float32), (128, 64)).copy()

    # idx wrapped for dma_gather, one wrap per GATHER GROUP (must mirror
    # gat_bass.gather_groups): elem i of group -> [16k + i%16, g0*8 + i//16]
    k_lo, k_hi = plan["k_lo"], plan["k_hi"]
    gb_sz = plan.get("gather_batch", 1)
    groups = []
    for base, cnt in ((0, k_lo), (k_lo, k_hi)):
        j = 0
        while j < cnt:
            n = min(gb_sz, cnt - j)
            groups.append((base + j, n))
            j += n
    idx_in = np.zeros((n_cores, 128, n_chunks * 8), np.int16)
    seg_in = np.zeros((n_cores, 128, n_chunks), np.float32)
    for c in range(n_cores):
        for blk in range(n_blocks):
            gb = c * n_blocks + blk
            for (j0, ng) in groups:
                flat = plan["idx16"][gb, j0 * P:(j0 + ng) * P]
                wrap = flat.reshape(ng * 8, 16).T     # [16, ng*8]
                g0 = blk * K + j0
                idx_in[c, :, g0 * 8:g0 * 8 + ng * 8] = np.tile(wrap, (8, 1))
            for j in range(K):
                g = blk * K + j
                seg_in[c, :, g] = plan["seg"][gb, j * P:(j + 1) * P]
    iotap = np.arange(128, dtype=np.float32)[:, None].copy()
    iotakf = np.broadcast_to(
        np.tile(np.arange(128, dtype=np.float32), K), (128, K * 128)).copy()
    ins = []
    for c in range(n_cores):
        ins.append({
            "x": xp[c * NPC:(c + 1) * NPC].copy(),
            "idx": idx_in[c],
            "seg": seg_in[c],
            "wcat1": w1, "wcat2": w2, "wcat3": w3,
            "bias1": b1, "bias2": b2, "bias3": b3,
            "iden": np.eye(128, dtype=np.float32),
            "iotap": iotap,
            "iotakf": iotakf,
            "asrc1": _asrc_flat(weights["a_src1"], BF),
            "asrc2": _asrc_flat(weights["a_src2"], BF),
            "asrc3": _asrc_flat(weights["a_src3"], np.float32),
        })
    return ins


def _asrc_flat(a, dt):
    flat = np.asarray(a, np.float32).reshape(-1)
    return np.broadcast_to(flat, (128, flat.shape[0])).astype(dt).copy()


def unpack_output(plan, outs, n_nodes=50000):
    perm, NP = plan["perm"], plan["NP"]
    full = np.concatenate([o["out"] for o in outs], axis=0)
    assert full.shape[0] == NP
    valid = perm >= 0
    res = np.zeros((n_nodes, full.shape[1]), np.float32)
    res[perm[valid]] = full[valid]
    return res


F32 = mybir.dt.float32
BF16 = mybir.dt.bfloat16
AF = mybir.ActivationFunctionType
OP = mybir.AluOpType


def gather_groups(k_lo, k_hi, gb):
    groups = []
    for base, cnt in ((0, k_lo), (k_lo, k_hi)):
        j = 0
        while j < cnt:
            n = min(gb, cnt - j)
            groups.append((base + j, n, base == 0))
            j += n
    return groups


USE_LRELU = False
SKIP_AG = False      # timing-probe only: drop the AllGathers
SKIP_GATHER = False  # timing-probe only: drop the dma_gathers


def build_gat_v2(n_blocks: int, k_lo: int, k_hi: int, n_cores: int = 8,
                 in_feat: int = 256, gather_batch: int = 9,
                 ag_groups: int = 1):
    """Input tensor names (per core):
      x      [NPC, in_feat] bf16
      idx    [128, n_chunks*8] int16  (wrapped+replicated dma_gather indices)
      seg    [128, n_chunks] f32      (local dst 0..127, -1 pad)
      wcat1  [128, in_feat//128, 272] bf16; wcat2 [128,2,272]; wcat3 [128,2,66]
      asrc1, asrc2 [128, 256] bf16 replicated a_src flat; asrc3 [128, 64] f32
      bias1, bias2 [128, 256] f32; bias3 [128, 64] f32
      iden [128,128] f32; iotap [128,1] f32; iotakf [128, K*128] f32
    Output: out [NPC, 64] f32
    """
    P = 128
    K = k_lo + k_hi
    NPC = n_blocks * P
    NP = NPC * n_cores
    half = NP // 2
    n_chunks = n_blocks * K
    assert n_blocks % ag_groups == 0
    grp_blocks = n_blocks // ag_groups

    nc = bacc.Bacc("TRN2", target_bir_lowering=False, debug=False,
                   num_devices=n_cores)

    x_in = nc.dram_tensor("x", [NPC, in_feat], BF16, kind="ExternalInput")
    idx_in = nc.dram_tensor("idx", [128, n_chunks * 8], mybir.dt.int16,
                            kind="ExternalInput")
    seg_in = nc.dram_tensor("seg", [128, n_chunks], F32, kind="ExternalInput")
    kt1 = in_feat // 128
    wcat_in = [
        nc.dram_tensor("wcat1", [128, kt1, 272], BF16, kind="ExternalInput"),
        nc.dram_tensor("wcat2", [128, 2, 272], BF16, kind="ExternalInput"),
        nc.dram_tensor("wcat3", [128, 2, 66], BF16, kind="ExternalInput"),
    ]
    asrc_in = [
        nc.dram_tensor("asrc1", [128, 256], BF16, kind="ExternalInput"),
        nc.dram_tensor("asrc2", [128, 256], BF16, kind="ExternalInput"),
        nc.dram_tensor("asrc3", [128, 64], F32, kind="ExternalInput"),
    ]
    bias_in = [
        nc.dram_tensor("bias1", [128, 256], F32, kind="ExternalInput"),
        nc.dram_tensor("bias2", [128, 256], F32, kind="ExternalInput"),
        nc.dram_tensor("bias3", [128, 64], F32, kind="ExternalInput"),
    ]
    iden_in = nc.dram_tensor("iden", [128, 128], F32, kind="ExternalInput")
    iotap_in = nc.dram_tensor("iotap", [128, 1], F32, kind="ExternalInput")
    iotakf_in = nc.dram_tensor("iotakf", [128, K * 128], F32,
                               kind="ExternalInput")
    out_ext = nc.dram_tensor("out", [NPC, 64], F32, kind="ExternalOutput")

    # (F, H, C, in_feat, table dtype)
    LAY = [(256, 8, 32, in_feat, BF16), (256, 8, 32, 256, BF16),
           (64, 1, 64, 256, F32)]

    T_loc = [nc.dram_tensor(f"Tloc{i}", [NPC, LAY[i][0]], LAY[i][4])
             for i in range(3)]
    T_full = [nc.dram_tensor(f"Tfull{i}", [NP, LAY[i][0]], LAY[i][4],
                             addr_space="Shared")
              for i in range(3)]
    act_d = [None,
             nc.dram_tensor("act2", [NPC, 256], BF16),
             nc.dram_tensor("act3", [NPC, 256], BF16)]

    replica_groups = [list(range(n_cores))]

    with ExitStack() as ctx:
        tc = ctx.enter_context(tile.TileContext(nc))
        const = ctx.enter_context(tc.tile_pool(name="const", bufs=1))
        sb = ctx.enter_context(tc.tile_pool(name="sb", bufs=2))
        sb3 = ctx.enter_context(tc.tile_pool(name="sb3", bufs=3))
        ps = ctx.enter_context(tc.tile_pool(name="ps", bufs=2, space="PSUM"))

        nc.gpsimd.load_library(mlp)

        ident = const.tile([128, 128], F32)
        nc.sync.dma_start(out=ident[:], in_=iden_in[:])
        ident_b = const.tile([128, 128], BF16)
        nc.vector.tensor_copy(out=ident_b[:], in_=ident[:])
        iotap = const.tile([128, 1], F32)
        nc.sync.dma_start(out=iotap[:], in_=iotap_in[:])
        iotakf = const.tile([128, K * 128], F32)
        nc.sync.dma_start(out=iotakf[:], in_=iotakf_in[:])
        idx_t = const.tile([128, n_chunks * 8], mybir.dt.int16)
        nc.sync.dma_start(out=idx_t[:], in_=idx_in[:])
        seg_t = const.tile([128, n_chunks], F32)
        nc.sync.dma_start(out=seg_t[:], in_=seg_in[:])

        for li in range(3):
            F, H, C, IF, TDl = LAY[li]
            KT = IF // 128
            C2 = F + 2 * H
            KH = (K + 1) * H
            act_ap = x_in if li == 0 else act_d[li]
            Tf = T_full[li]
            Tf3 = Tf[:].rearrange("(r n) c -> r n c", r=n_cores)

            # ---------------- per-layer constants
            wc = sb.tile([128, KT, C2], BF16, tag="wc")
            nc.sync.dma_start(out=wc[:], in_=wcat_in[li][:])
            bia = sb.tile([128, F], F32, tag="bias")
            nc.sync.dma_start(out=bia[:], in_=bias_in[li][:])
            asr = sb.tile([128, F], TDl, tag="asrc")
            nc.sync.dma_start(out=asr[:], in_=asrc_in[li][:])
            asr_rep = sb.tile([128, K + 1, F], TDl, tag="asrep")
            nc.vector.tensor_copy(
                out=asr_rep[:],
                in_=asr[:, None, :].to_broadcast([128, K + 1, F]))
            al_d_all = sb.tile([128, n_blocks, H], TDl, tag=f"ald{li}")

            # ---------------- dense phase + interleaved AllGather
            for rb in range(n_blocks):
                att = sb3.tile([128, KT, 128], BF16, tag="att")
                for kt in range(KT):
                    nc.scalar.dma_start_transpose(
                        out=att[:, kt, :],
                        in_=act_ap[rb * P:(rb + 1) * P, kt * 128:(kt + 1) * 128])
                pd = ps.tile([128, C2], F32, tag="pdense")
                for kt in range(KT):
                    nc.tensor.matmul(pd[:], lhsT=att[:, kt, :], rhs=wc[:, kt, :],
                                     start=(kt == 0), stop=(kt == KT - 1))
                trow = sb3.tile([128, F], TDl, tag="trow")
                nc.vector.tensor_copy(out=trow[:], in_=pd[:, :F])
                nc.sync.dma_start(out=T_loc[li][rb * P:(rb + 1) * P, :],
                                  in_=trow[:])
                nc.vector.tensor_copy(out=al_d_all[:, rb, :], in_=pd[:, F + H:])
                if (rb + 1) % grp_blocks == 0 and not SKIP_AG:
                    r0 = (rb + 1 - grp_blocks) * P
                    r1 = (rb + 1) * P
                    # AllGather straight into the gather table: with the
                    # core-major [r, n, c] slab layout, one group spanning the
                    # whole local table IS the natural table order — no
                    # un-interleave copies needed.
                    nc.gpsimd.collective_compute(
                        "AllGather", OP.bypass,
                        replica_groups=replica_groups,
                        ins=[T_loc[li][r0:r1, :]],
                        outs=[Tf3[:, r0:r1, :]],
                    )

            # ---------------- edge phase
            for b in range(n_blocks):
                G_all = sb.tile([128, K + 1, F], TDl, tag="G")
                for gi, (j0, ng, is_lo) in enumerate(
                        [] if SKIP_GATHER else
                        gather_groups(k_lo, k_hi, gather_batch)):
                    g0 = b * K + j0
                    src_view = Tf[:half, :] if is_lo else Tf[half:, :]
                    nc.gpsimd.dma_gather(
                        out_ap=G_all[:, j0:j0 + ng, :],
                        in_ap=src_view,
                        idxs_ap=idx_t[:, g0 * 8:g0 * 8 + ng * 8],
                        num_idxs=ng * 128,
                        num_idxs_reg=ng * 128,
                        elem_size=F,
                        queue_num=0,
                    )
                # self rows as chunk K
                nc.sync.dma_start(out=G_all[:, K, :],
                                  in_=T_loc[li][b * P:(b + 1) * P, :])

                # S (edge-major one-hot) for all K chunks in one op
                S_all = sb.tile([128, K, 128], TDl, tag="S")
                nc.vector.tensor_tensor(
                    out=S_all[:],
                    in0=iotakf[:].rearrange("p (k d) -> p k d", k=K),
                    in1=seg_t[:, b * K:(b + 1) * K, None].to_broadcast(
                        [128, K, 128]),
                    op=OP.is_equal)

                # Sd (dst-major one-hot) via transpose trick, 4 chunks/bank;
                # ald matmuls accumulate into one psum tile
                ald_ps = ps.tile([128, K * H], F32, tag="aldps")
                for g4 in range(0, K, 4):
                    n4 = min(4, K - g4)
                    pt = ps.tile([128, 4, 128], F32, tag="ptr")
                    for i in range(n4):
                        g = b * K + g4 + i
                        nc.tensor.transpose(
                            pt[:, i, :],
                            seg_t[:, g:g + 1].to_broadcast([128, 128]),
                            ident[:])
                    Sd4 = sb3.tile([128, 4, 128], TDl, tag="Sd")
                    nc.vector.tensor_scalar(
                        out=Sd4[:, :n4, :], in0=pt[:, :n4, :],
                        scalar1=iotap[:, :], scalar2=None, op0=OP.is_equal)
                    for i in range(n4):
                        j = g4 + i
                        nc.tensor.matmul(
                            ald_ps[:, j * H:(j + 1) * H],
                            lhsT=Sd4[:, i, :], rhs=al_d_all[:, b, :],
                            start=True, stop=True)

                ald_all = sb3.tile([128, KH], F32, tag="alda")
                nc.vector.tensor_copy(out=ald_all[:, :K * H], in_=ald_ps[:])
                nc.vector.tensor_copy(out=ald_all[:, K * H:],
                                      in_=al_d_all[:, b, :])

                # al_s for all chunks (incl self) in two ops
                tmp = sb3.tile([128, (K + 1) * F], TDl, tag="alstmp")
                nc.vector.tensor_tensor(
                    out=tmp[:], in0=G_all[:].rearrange("p k f -> p (k f)"),
                    in1=asr_rep[:].rearrange("p k f -> p (k f)"), op=OP.mult)
                als_all = sb3.tile([128, KH], F32, tag="alsa")
                nc.vector.tensor_reduce(
                    out=als_all[:],
                    in_=tmp[:].rearrange("p (g c) -> p g c", c=C),
                    axis=mybir.AxisListType.X, op=OP.add)

                # logits -> leaky relu -> exp  (batched)
                lg = sb3.tile([128, KH], F32, tag="lg")
                nc.vector.tensor_tensor(out=lg[:], in0=ald_all[:],
                                        in1=als_all[:], op=OP.add)
                lt = sb3.tile([128, KH], F32, tag="lt")
                if USE_LRELU:
                    nc.scalar.activation(out=lt[:], in_=lg[:], func=AF.Lrelu,
                                         alpha=0.2)
                else:
                    nc.vector.tensor_scalar_mul(lt[:], lg[:], 0.2)
                    nc.vector.tensor_tensor(out=lt[:], in0=lt[:], in1=lg[:],
                                            op=OP.max)
                exb = sb3.tile([128, KH], TDl, tag="exb")
                nc.scalar.activation(out=exb[:], in_=lt[:], func=AF.Exp)

                # rhs = [G*ex | ex]
                rhs = sb.tile([128, K + 1, H, C + 1], TDl, tag="rhs")
                exv = exb[:].rearrange("p (k h) -> p k h", h=H)
                nc.vector.tensor_tensor(
                    out=rhs[:, :, :, :C],
                    in0=G_all[:].rearrange("p k (h c) -> p k h c", h=H),
                    in1=exv[:, :, :, None].to_broadcast([128, K + 1, H, C]),
                    op=OP.mult)
                nc.vector.tensor_copy(out=rhs[:, :, :, C], in_=exv)

                # aggregation
                pa = ps.tile([128, H * (C + 1)], F32, tag="pagg")
                for j in range(K + 1):
                    if j < K:
                        lhs_agg = S_all[:, j, :]
                    else:
                        lhs_agg = ident_b[:] if TDl == BF16 else ident[:]
                    nc.tensor.matmul(
                        pa[:], lhsT=lhs_agg,
                        rhs=rhs[:, j].rearrange("p h c -> p (h c)"),
                        start=(j == 0), stop=(j == K))

                # normalize + bias (+ELU)
                pa3 = pa[:].rearrange("p (h c) -> p h c", h=H)
                dn = sb3.tile([128, H], F32, tag="dn")
                nc.vector.tensor_scalar_add(dn[:], pa3[:, :, C], 1e-30)
                rc = sb3.tile([128, H], F32, tag="rc")
                nc.vector.reciprocal(rc[:], dn[:])
                ob = sb3.tile([128, F], F32, tag="ob")
                ob3 = ob[:].rearrange("p (h c) -> p h c", h=H)
                nc.vector.tensor_tensor(out=ob3, in0=pa3[:, :, :C],
                                        in1=rc[:, :, None].to_broadcast([128, H, C]),
                                        op=OP.mult)
                nc.vector.tensor_tensor(out=ob[:], in0=ob[:], in1=bia[:],
                                        op=OP.add)
                if li < 2:
                    # elu(x) = max(x,0) - 1 + exp(min(x,0))
                    mn = sb3.tile([128, F], F32, tag="mn")
                    nc.vector.tensor_scalar_min(mn[:], ob[:], 0.0)
                    em = sb3.tile([128, F], F32, tag="em")
                    nc.scalar.activation(out=em[:], in_=mn[:], func=AF.Exp)
                    acb = sb3.tile([128, F], F32, tag="acb")
                    nc.vector.tensor_scalar(
                        out=acb[:], in0=ob[:], scalar1=0.0, scalar2=-1.0,
                        op0=OP.max, op1=OP.add)
                    actb = sb3.tile([128, F], BF16, tag="actb")
                    nc.vector.tensor_tensor(out=actb[:], in0=acb[:], in1=em[:],
                                            op=OP.add)
                    nc.sync.dma_start(
                        out=act_d[li + 1][b * P:(b + 1) * P, :], in_=actb[:])
                else:
                    nc.sync.dma_start(out=out_ext[b * P:(b + 1) * P, :],
                                      in_=ob[:])

    nc.compile()
    return nc


class SpmdRunner:
    def __init__(self, nc, n_cores: int):
        install_neuronx_cc_hook()
        self.nc = nc
        self.n_cores = n_cores
        in_names, out_names, out_avals, zero_outs = [], [], [], []
        partition_name = nc.partition_id_tensor.name if nc.partition_id_tensor else None
        for alloc in nc.m.functions[0].allocations:
            if not isinstance(alloc, mybir.MemoryLocationSet):
                continue
            name = alloc.memorylocations[0].name
            if alloc.kind == "ExternalInput":
                if name != partition_name:
                    in_names.append(name)
            elif alloc.kind == "ExternalOutput":
                out_names.append(name)
                shape = tuple(alloc.tensor_shape)
                dtype = mybir.dt.np(alloc.dtype)
                out_avals.append(jax.core.ShapedArray(shape, dtype))
                zero_outs.append(np.zeros(shape, dtype))
        self.in_names, self.out_names = in_names, out_names
        self.out_avals, self.zero_outs = out_avals, zero_outs
        n_params = len(in_names)
        n_outs = len(out_avals)
        all_in_names = list(in_names) + list(out_names)
        if partition_name is not None:
            all_in_names.append(partition_name)

        def _body(*args):
            operands = list(args)
            if partition_name is not None:
                operands.append(bass2jax.partition_id_tensor())
            outs = _bass_exec_p.bind(
                *operands,
                out_avals=tuple(out_avals),
                in_names=tuple(all_in_names),
                out_names=tuple(out_names),
                lowering_input_output_aliases=(),
                sim_require_finite=True,
                sim_require_nnan=True,
                nc=nc,
            )
            return tuple(outs)

        devices = jax.devices()[:n_cores]
        self.mesh = Mesh(np.asarray(devices), ("core",))
        in_specs = (PartitionSpec("core"),) * (n_params + n_outs)
        out_specs = (PartitionSpec("core"),) * n_outs
        self.fn = jax.jit(
            shard_map(_body, mesh=self.mesh, in_specs=in_specs,
                      out_specs=out_specs, check_rep=False),
            keep_unused=True,
        )
        self.dev_in = None

    def set_inputs(self, in_maps):
        concat_in = [
            np.concatenate([np.asarray(in_maps[c][name]) for c in range(self.n_cores)], axis=0)
            for name in self.in_names
        ]
        concat_zeros = [
            np.zeros((self.n_cores * z.shape[0], *z.shape[1:]), z.dtype)
            for z in self.zero_outs
        ]
        sharding = jax.sharding.NamedSharding(self.mesh, PartitionSpec("core"))
        self.dev_in = [jax.device_put(a, sharding) for a in concat_in + concat_zeros]

    def __call__(self):
        outs = self.fn(*self.dev_in)
        jax.block_until_ready(outs)
        return outs

    def results(self, outs):
        per_core = []
        for c in range(self.n_cores):
            d = {}
            for i, name in enumerate(self.out_names):
                full = np.asarray(outs[i])
                sh = self.out_avals[i].shape
                d[name] = full.reshape(self.n_cores, *sh)[c]
            per_core.append(d)
        return per_core


# ======================================================================
# kernel() entry point
# ======================================================================

_CACHE = {}
_RUNNER = {}

N_NODES = 50000
K_TRY = [(9, 9), (10, 10), (12, 12)]
GATHER_BATCH = 3
AG_GROUPS = 1


def _get_compiled(n_blocks, k_lo, k_hi):
    key = (n_blocks, k_lo, k_hi)
    if key not in _CACHE:
        _CACHE[key] = build_gat_v2(n_blocks=n_blocks, k_lo=k_lo, k_hi=k_hi,
                                   gather_batch=min(GATHER_BATCH, k_lo),
                                   ag_groups=AG_GROUPS if n_blocks % AG_GROUPS == 0 else 1)
    return _CACHE[key]


def _plan_any(edge_index, n_nodes):
    last = None
    for k_lo, k_hi in K_TRY:
        try:
            plan = plan_graph(edge_index, n_nodes=n_nodes, k_lo=k_lo, k_hi=k_hi)
            plan["gather_batch"] = min(GATHER_BATCH, k_lo)
            return plan
        except RuntimeError as e:
            last = e
    raise last


def kernel(**inputs):
    x = np.asarray(inputs["x"])
    edge_index = np.asarray(inputs["edge_index"])
    n_nodes = x.shape[0]
    weights = {k: np.asarray(v) for k, v in inputs.items()
               if k not in ("x", "edge_index")}

    plan = _plan_any(edge_index, n_nodes)
    ins = pack_inputs(plan, x, weights)

    key = (plan["n_blocks"], plan["k_lo"], plan["k_hi"])
    nc = _get_compiled(*key)
    if key not in _RUNNER:
        _RUNNER[key] = SpmdRunner(nc, 8)
    r = _RUNNER[key]
    r.set_inputs(ins)
    outs = r()
    res = unpack_output(plan, r.results(outs), n_nodes=n_nodes)
    return res.astype(np.float32)

